# revision 22
# baseline (speedup 1.0000x reference)
"""Trainium2 Bass kernel for the entmax-bisect Tsallis loss (nn_BisectionLoss).

Math: the reference runs a 50-step f32 bisection on
f(t) = sum(relu(Xs - t)^(1/(V-1))) - 1 with Xs = 0.5*X.  Because the exponent
1/(V-1) is tiny, every element above t contributes ~1 and the rest contribute
0, so the bisection decision at every step is [x2 > t] (x2 = second-largest).
The limit is t* = min(x2, t_max) with t_max = m - V^(1-alpha):
  - gap(1,2) >= V^(1-alpha): t converges one-f32-ulp below x2;
  - gap(1,2) <  V^(1-alpha): t converges to t_max (support can then hold any
    element within V^(1-alpha) of the max; measured max support is 3).
The loss is insensitive to t at the 1e-5 level (it enters only through
(x-t)^(1/31999)), so instead of iterating we set t directly:
  t = min(x2 - 2.4e-7, m - V^(1-alpha))
which reproduces the bisection's t within one ulp and (verified numerically
on the input distribution) never flips a support-membership decision.
Host-side emulation vs the exact-bisection reference: max rel err 3.6e-6;
measured on HW: 4.2e-6.

Device work per core (memory-bound, one pass over X):
  1. Stream X in [128, w] chunks (8000 wide, 5 buffers: the deep window
     keeps the DMA rings saturated despite the in-loop transfer latency)
     on two HWDGE rings.  The scalar/ACT ring carries only EARLY chunks,
     so every dma_start on the ACT queue issues in the first half of the
     stream and the finalize activations behind them never block a pending
     transfer.  The sync ring carries the rest, including the tapered last
     row-tile (8000..1000) so the final Max8 catch-up is short.
  2. Per-tile finalize (~17 small ops): t as above, Z = relu(Xs-t)^eps via
     ACT ln/exp, p = Z/S, partial loss = dot(p, X_top8) + (Sa-1)/0.75.
     Finalize for tile j is issued after tile j+1's stream so its DVE ops
     never stall the chunk Max8 pipeline.  The X[row, target] subtraction
     happens on the host (which gathers those values anyway).
  3. Losses accumulate in a [128, NT] tile; a PE transpose through PSUM
     re-partitions them to [NT, 128] so the OUT write is one DMA with four
     512-byte descriptors instead of 128 16-byte ones.
Sharding: rows split evenly across 8 cores; no communication.
"""

from contextlib import ExitStack

import numpy as np

B, V = 4096, 32000
NCORES = 8
RB = B // NCORES  # 512 rows per core
P = 128
NT = RB // P  # 4 row-tiles per core
ALPHA = 1.5
EPS = np.float32(1.0 / (V - 1))
CVAL = np.float32(V ** (1.0 - ALPHA))
INV_DENOM = np.float32(1.0 / (ALPHA * (ALPHA - 1.0)))  # 1/0.75
DSTAR = np.float32(2.4e-7)  # ~1 ulp at x2~2; keeps t strictly below x2

# Per-row-tile chunk widths.  Tiles 0-2 stream in two big chunks; the last
# tile tapers so the tail Max8 catch-up after the final DMA is tiny.
PLAN = [
    [2000, 2000, 4000, 8000, 8000, 8000],
    [8000, 8000, 8000, 8000],
    [8000, 8000, 8000, 8000],
    [8000, 8000, 6000, 4000, 2000, 2000, 1000, 1000],
]
assert all(sum(p) == V for p in PLAN) and len(PLAN) == NT
SCALAR_CHUNKS = {1, 3, 5, 7, 9, 11, 13}  # early chunks carried by the ACT ring

_CACHE: dict = {}


def _build():
    import concourse.bass as bass  # noqa: F401
    import concourse.tile as tile
    from concourse import bacc, masks, mybir

    f32 = mybir.dt.float32
    AX = mybir.AxisListType.X
    Alu = mybir.AluOpType
    Act = mybir.ActivationFunctionType

    nc = bacc.Bacc(
        "TRN2", target_bir_lowering=False, debug=False, enable_asserts=False
    )
    Xp = nc.declare_dram_parameter("X", [RB, V], f32, isOutput=False)
    OUTp = nc.declare_dram_parameter("OUT", [RB], f32, isOutput=True)
    X = Xp.ap()

    nch = sum(len(p) for p in PLAN)

    with tile.TileContext(nc) as tc, ExitStack() as ctx:
        xpool = ctx.enter_context(tc.tile_pool(name="xc", bufs=5))
        sp = ctx.enter_context(tc.tile_pool(name="small", bufs=1))
        pp = ctx.enter_context(tc.tile_pool(name="ps", bufs=1, space="PSUM"))

        cand = sp.tile([P, nch * 8], f32)
        top8 = sp.tile([P, NT * 8], f32)
        lossT = sp.tile([P, NT], f32)
        lossF = sp.tile([NT, P], f32)
        ident = sp.tile([P, P], f32)
        masks.make_identity(nc, ident[:])

        cseq = [0]  # global chunk counter (ring assignment)
        coff = [0]  # global candidate-slot offset

        def stream_tile(j, mid=None):
            k0 = coff[0]
            col = 0
            for ci, w in enumerate(PLAN[j]):
                xt_ = xpool.tile([P, w], f32, tag="xc")
                eng = nc.scalar if cseq[0] in SCALAR_CHUNKS else nc.sync
                eng.dma_start(xt_[:], X[j * P : (j + 1) * P, col : col + w])
                k = coff[0] * 8
                nc.vector.max(cand[:, k : k + 8], xt_[:])
                cseq[0] += 1
                coff[0] += 1
                col += w
                if mid is not None and ci == 1:
                    mid()
            nc.vector.max(
                top8[:, j * 8 : (j + 1) * 8],
                cand[:, k0 * 8 : coff[0] * 8],
            )

        def finalize(jlo, jhi):
            """Direct-threshold partial loss for row-tiles [jlo, jhi)."""
            n = jhi - jlo
            w = n * 8
            t8 = top8[:, jlo * 8 : jhi * 8]  # [P, w] X-space top-8
            Xs = sp.tile([P, w], f32, tag=f"xs{jlo}")
            nc.vector.tensor_scalar_mul(Xs[:], t8, 0.5)
            m = Xs[:][:, 0:w:8]  # [P, n]
            x2 = Xs[:][:, 1:w:8]
            tmax = sp.tile([P, n], f32, tag=f"tm{jlo}")
            t = sp.tile([P, n], f32, tag=f"t{jlo}")
            nc.vector.tensor_scalar_sub(tmax[:], m, float(CVAL))
            nc.vector.tensor_scalar_sub(t[:], x2, float(DSTAR))
            nc.vector.tensor_tensor(t[:], t[:], tmax[:], Alu.min)

            xs3 = Xs[:].rearrange("p (j k) -> p j k", k=8)
            v3 = t8.rearrange("p (j k) -> p j k", k=8)
            tb = t[:].rearrange("p (j one) -> p j one", one=1).broadcast_to([P, n, 8])
            u = sp.tile([P, n, 8], f32, tag=f"u{jlo}")
            nc.vector.scalar_tensor_tensor(
                out=u[:], in0=xs3, scalar=1.0, in1=tb,
                op0=Alu.mult, op1=Alu.subtract,
            )
            msk = sp.tile([P, n, 8], f32, tag=f"mk{jlo}")
            nc.vector.tensor_scalar(
                out=msk[:], in0=u[:], scalar1=0.0, scalar2=None, op0=Alu.is_gt
            )
            nc.vector.tensor_scalar_max(u[:], u[:], 1e-38)
            nc.scalar.activation(u[:], u[:], Act.Ln)
            nc.scalar.activation(u[:], u[:], Act.Exp, scale=float(EPS))
            Z = sp.tile([P, n, 8], f32, tag=f"z{jlo}")
            nc.vector.tensor_mul(Z[:], u[:], msk[:])
            S1 = sp.tile([P, n], f32, tag=f"s1{jlo}")
            nc.vector.reduce_sum(
                S1[:].rearrange("p (j one) -> p j one", one=1), Z[:], axis=AX
            )
            rcp = sp.tile([P, n], f32, tag=f"rc{jlo}")
            nc.vector.reciprocal(rcp[:], S1[:])
            rb = rcp[:].rearrange("p (j one) -> p j one", one=1).broadcast_to([P, n, 8])
            p = sp.tile([P, n, 8], f32, tag=f"p{jlo}")
            nc.vector.scalar_tensor_tensor(
                out=p[:], in0=Z[:], scalar=1.0, in1=rb,
                op0=Alu.mult, op1=Alu.mult,
            )
            sq = sp.tile([P, n, 8], f32, tag=f"sq{jlo}")
            nc.scalar.activation(sq[:], p[:], Act.Sqrt)
            nc.vector.tensor_mul(sq[:], p[:], sq[:])  # p^1.5
            Sa = sp.tile([P, n], f32, tag=f"sa{jlo}")
            nc.vector.reduce_sum(
                Sa[:].rearrange("p (j one) -> p j one", one=1), sq[:], axis=AX
            )
            q = sp.tile([P, n], f32, tag=f"q{jlo}")
            nc.vector.tensor_scalar(
                out=q[:], in0=Sa[:], scalar1=1.0, scalar2=float(INV_DENOM),
                op0=Alu.subtract, op1=Alu.mult,
            )  # (Sa-1)/0.75 == -(1-Sa)/0.75
            nc.vector.tensor_mul(p[:], p[:], v3)  # p * X_top8
            D = sp.tile([P, n], f32, tag=f"dd{jlo}")
            nc.vector.reduce_sum(
                D[:].rearrange("p (j one) -> p j one", one=1), p[:], axis=AX
            )
            nc.vector.tensor_sub(lossT[:, jlo:jhi], D[:], q[:])

        # Finalize for tile j is issued after tile j+1's stream: its ACT ops
        # then sit behind already-issuable dma_starts on the ACT queue, and
        # its DVE ops never make a chunk Max8 wait on an ACT result.
        stream_tile(0)
        stream_tile(1)
        finalize(0, 1)
        stream_tile(2)
        finalize(1, 2)
        stream_tile(3)
        finalize(2, 4)

        # Re-partition losses via PE transpose so the OUT write has four
        # 512-byte descriptors (one per row-tile) instead of 128 tiny ones.
        pbank = pp.tile([P, nc.PSUM_BANK_SIZE_BYTES // 4], f32)
        nc.tensor.transpose(pbank[:][:NT, :P], lossT[:], ident[:])
        nc.vector.tensor_copy(lossF[:], pbank[:][:NT, :P])
        nc.scalar.dma_start(
            OUTp.ap().rearrange("(j p) -> j p", p=P), lossF[:]
        )

    nc.compile()
    return nc


def get_nc():
    if "nc" not in _CACHE:
        _CACHE["nc"] = _build()
    return _CACHE["nc"]


def make_in_maps(X, target):
    return [{"X": X[c * RB : (c + 1) * RB]} for c in range(NCORES)]


def post(raw, X, target):
    """Host-side finish: subtract the gathered X[row, target] values."""
    xt = X[np.arange(B), target.astype(np.int64)].astype(np.float32)
    return raw - xt


def kernel(X: np.ndarray, target: np.ndarray) -> np.ndarray:
    from concourse.bass_utils import run_bass_kernel_spmd

    X = np.ascontiguousarray(np.asarray(X, dtype=np.float32))
    target = np.asarray(target)
    assert X.shape == (B, V) and target.shape == (B,)

    nc = get_nc()
    res = run_bass_kernel_spmd(
        nc, make_in_maps(X, target), core_ids=list(range(NCORES))
    ).results
    raw = np.concatenate([res[c]["OUT"] for c in range(NCORES)], axis=0)
    return post(raw, X, target)


# revision 23
# speedup vs baseline: 1.1491x; 1.1491x over previous
"""Trainium2 Bass kernel for the entmax-bisect Tsallis loss (nn_BisectionLoss).

Math: the reference runs a 50-step f32 bisection on
f(t) = sum(relu(Xs - t)^(1/(V-1))) - 1 with Xs = 0.5*X.  Because the exponent
1/(V-1) is tiny, every element above t contributes ~1 and the rest contribute
0, so the bisection decision at every step is [x2 > t] (x2 = second-largest).
The limit is t* = min(x2, t_max) with t_max = m - V^(1-alpha):
  - gap(1,2) >= V^(1-alpha): t converges one-f32-ulp below x2;
  - gap(1,2) <  V^(1-alpha): t converges to t_max (support can then hold any
    element within V^(1-alpha) of the max; measured max support is 3).
The loss is insensitive to t at the 1e-5 level (it enters only through
(x-t)^(1/31999)), so instead of iterating we set t directly:
  t = min(x2 - 2.4e-7, m - V^(1-alpha))
which reproduces the bisection's t within one ulp and (verified numerically
on the input distribution) never flips a support-membership decision.
Host-side emulation vs the exact-bisection reference: max rel err 3.6e-6;
measured on HW: 4.2e-6.

Device work per core (memory-bound, one pass over X):
  1. Stream X in [128, w] chunks (8000 wide, 5 buffers: the deep window
     keeps the DMA rings saturated despite the in-loop transfer latency)
     on two HWDGE rings.  The scalar/ACT ring carries only EARLY chunks,
     so every dma_start on the ACT queue issues in the first half of the
     stream and the finalize activations behind them never block a pending
     transfer.  The sync ring carries the rest, including the tapered last
     row-tile (8000..1000) so the final Max8 catch-up is short.
  2. Per-tile finalize (~17 small ops): t as above, Z = relu(Xs-t)^eps via
     ACT ln/exp, p = Z/S, partial loss = dot(p, X_top8) + (Sa-1)/0.75.
     Finalize for tile j is issued after tile j+1's stream so its DVE ops
     never stall the chunk Max8 pipeline.  The X[row, target] subtraction
     happens on the host (which gathers those values anyway).
  3. Losses accumulate in a [128, NT] tile; a PE transpose through PSUM
     re-partitions them to [NT, 128] so the OUT write is one DMA with four
     512-byte descriptors instead of 128 16-byte ones.
Sharding: rows split evenly across 8 cores; no communication.
"""

from contextlib import ExitStack

import numpy as np

B, V = 4096, 32000
NCORES = 8
RB = B // NCORES  # 512 rows per core
P = 128
NT = RB // P  # 4 row-tiles per core
ALPHA = 1.5
EPS = np.float32(1.0 / (V - 1))
CVAL = np.float32(V ** (1.0 - ALPHA))
INV_DENOM = np.float32(1.0 / (ALPHA * (ALPHA - 1.0)))  # 1/0.75
DSTAR = np.float32(2.4e-7)  # ~1 ulp at x2~2; keeps t strictly below x2

# Per-row-tile chunk widths.  Tiles 0-2 stream in two big chunks; the last
# tile tapers so the tail Max8 catch-up after the final DMA is tiny.
PLAN = [
    [8000, 8000, 8000, 8000],
    [8000, 8000, 8000, 8000],
    [8000, 8000, 8000, 8000],
    [8000, 8000, 6000, 4000, 2000, 2000, 1000, 1000],
]
assert all(sum(p) == V for p in PLAN) and len(PLAN) == NT
SCALAR_CHUNKS = {1, 3, 5, 7, 9, 11}  # early chunks carried by the ACT ring

_CACHE: dict = {}


def _build():
    import concourse.bass as bass  # noqa: F401
    import concourse.tile as tile
    from concourse import bacc, masks, mybir

    f32 = mybir.dt.float32
    AX = mybir.AxisListType.X
    Alu = mybir.AluOpType
    Act = mybir.ActivationFunctionType

    nc = bacc.Bacc(
        "TRN2", target_bir_lowering=False, debug=False, enable_asserts=False
    )
    Xp = nc.declare_dram_parameter("X", [RB, V], f32, isOutput=False)
    OUTp = nc.declare_dram_parameter("OUT", [RB], f32, isOutput=True)
    X = Xp.ap()

    nch = sum(len(p) for p in PLAN)

    with tile.TileContext(nc) as tc, ExitStack() as ctx:
        xpool = ctx.enter_context(tc.tile_pool(name="xc", bufs=5))
        sp = ctx.enter_context(tc.tile_pool(name="small", bufs=1))
        pp = ctx.enter_context(tc.tile_pool(name="ps", bufs=1, space="PSUM"))

        cand = sp.tile([P, nch * 8], f32)
        top8 = sp.tile([P, NT * 8], f32)
        lossT = sp.tile([P, NT], f32)
        lossF = sp.tile([NT, P], f32)
        ident = sp.tile([P, P], f32)
        masks.make_identity(nc, ident[:])

        cseq = [0]  # global chunk counter (ring assignment)
        coff = [0]  # global candidate-slot offset

        def stream_tile(j, mid=None):
            k0 = coff[0]
            col = 0
            for ci, w in enumerate(PLAN[j]):
                xt_ = xpool.tile([P, w], f32, tag="xc")
                eng = nc.scalar if cseq[0] in SCALAR_CHUNKS else nc.sync
                eng.dma_start(xt_[:], X[j * P : (j + 1) * P, col : col + w])
                k = coff[0] * 8
                nc.vector.max(cand[:, k : k + 8], xt_[:])
                cseq[0] += 1
                coff[0] += 1
                col += w
                if mid is not None and ci == 1:
                    mid()
            nc.vector.max(
                top8[:, j * 8 : (j + 1) * 8],
                cand[:, k0 * 8 : coff[0] * 8],
            )

        def finalize(jlo, jhi):
            """Direct-threshold partial loss for row-tiles [jlo, jhi)."""
            n = jhi - jlo
            w = n * 8
            t8 = top8[:, jlo * 8 : jhi * 8]  # [P, w] X-space top-8
            Xs = sp.tile([P, w], f32, tag=f"xs{jlo}")
            nc.vector.tensor_scalar_mul(Xs[:], t8, 0.5)
            m = Xs[:][:, 0:w:8]  # [P, n]
            x2 = Xs[:][:, 1:w:8]
            tmax = sp.tile([P, n], f32, tag=f"tm{jlo}")
            t = sp.tile([P, n], f32, tag=f"t{jlo}")
            nc.vector.tensor_scalar_sub(tmax[:], m, float(CVAL))
            nc.vector.tensor_scalar_sub(t[:], x2, float(DSTAR))
            nc.vector.tensor_tensor(t[:], t[:], tmax[:], Alu.min)

            xs3 = Xs[:].rearrange("p (j k) -> p j k", k=8)
            v3 = t8.rearrange("p (j k) -> p j k", k=8)
            tb = t[:].rearrange("p (j one) -> p j one", one=1).broadcast_to([P, n, 8])
            u = sp.tile([P, n, 8], f32, tag=f"u{jlo}")
            nc.vector.scalar_tensor_tensor(
                out=u[:], in0=xs3, scalar=1.0, in1=tb,
                op0=Alu.mult, op1=Alu.subtract,
            )
            msk = sp.tile([P, n, 8], f32, tag=f"mk{jlo}")
            nc.vector.tensor_scalar(
                out=msk[:], in0=u[:], scalar1=0.0, scalar2=None, op0=Alu.is_gt
            )
            nc.vector.tensor_scalar_max(u[:], u[:], 1e-38)
            nc.scalar.activation(u[:], u[:], Act.Ln)
            nc.scalar.activation(u[:], u[:], Act.Exp, scale=float(EPS))
            Z = sp.tile([P, n, 8], f32, tag=f"z{jlo}")
            nc.vector.tensor_mul(Z[:], u[:], msk[:])
            S1 = sp.tile([P, n], f32, tag=f"s1{jlo}")
            nc.vector.reduce_sum(
                S1[:].rearrange("p (j one) -> p j one", one=1), Z[:], axis=AX
            )
            rcp = sp.tile([P, n], f32, tag=f"rc{jlo}")
            nc.vector.reciprocal(rcp[:], S1[:])
            rb = rcp[:].rearrange("p (j one) -> p j one", one=1).broadcast_to([P, n, 8])
            p = sp.tile([P, n, 8], f32, tag=f"p{jlo}")
            nc.vector.scalar_tensor_tensor(
                out=p[:], in0=Z[:], scalar=1.0, in1=rb,
                op0=Alu.mult, op1=Alu.mult,
            )
            sq = sp.tile([P, n, 8], f32, tag=f"sq{jlo}")
            nc.scalar.activation(sq[:], p[:], Act.Sqrt)
            nc.vector.tensor_mul(sq[:], p[:], sq[:])  # p^1.5
            Sa = sp.tile([P, n], f32, tag=f"sa{jlo}")
            nc.vector.reduce_sum(
                Sa[:].rearrange("p (j one) -> p j one", one=1), sq[:], axis=AX
            )
            q = sp.tile([P, n], f32, tag=f"q{jlo}")
            nc.vector.tensor_scalar(
                out=q[:], in0=Sa[:], scalar1=1.0, scalar2=float(INV_DENOM),
                op0=Alu.subtract, op1=Alu.mult,
            )  # (Sa-1)/0.75 == -(1-Sa)/0.75
            nc.vector.tensor_mul(p[:], p[:], v3)  # p * X_top8
            D = sp.tile([P, n], f32, tag=f"dd{jlo}")
            nc.vector.reduce_sum(
                D[:].rearrange("p (j one) -> p j one", one=1), p[:], axis=AX
            )
            nc.vector.tensor_sub(lossT[:, jlo:jhi], D[:], q[:])

        # Finalize for tile j is issued after tile j+1's stream: its ACT ops
        # then sit behind already-issuable dma_starts on the ACT queue, and
        # its DVE ops never make a chunk Max8 wait on an ACT result.
        stream_tile(0)
        stream_tile(1)
        finalize(0, 1)
        stream_tile(2)
        finalize(1, 2)
        stream_tile(3)
        finalize(2, 3)
        finalize(3, 4)

        # Re-partition losses via PE transpose so the OUT write has four
        # 512-byte descriptors (one per row-tile) instead of 128 tiny ones.
        pbank = pp.tile([P, nc.PSUM_BANK_SIZE_BYTES // 4], f32)
        nc.tensor.transpose(pbank[:][:NT, :P], lossT[:], ident[:])
        nc.vector.tensor_copy(lossF[:], pbank[:][:NT, :P])
        nc.scalar.dma_start(
            OUTp.ap().rearrange("(j p) -> j p", p=P), lossF[:]
        )

    nc.compile()
    return nc


def get_nc():
    if "nc" not in _CACHE:
        _CACHE["nc"] = _build()
    return _CACHE["nc"]


def make_in_maps(X, target):
    return [{"X": X[c * RB : (c + 1) * RB]} for c in range(NCORES)]


def post(raw, X, target):
    """Host-side finish: subtract the gathered X[row, target] values."""
    xt = X[np.arange(B), target.astype(np.int64)].astype(np.float32)
    return raw - xt


def kernel(X: np.ndarray, target: np.ndarray) -> np.ndarray:
    from concourse.bass_utils import run_bass_kernel_spmd

    X = np.ascontiguousarray(np.asarray(X, dtype=np.float32))
    target = np.asarray(target)
    assert X.shape == (B, V) and target.shape == (B,)

    nc = get_nc()
    res = run_bass_kernel_spmd(
        nc, make_in_maps(X, target), core_ids=list(range(NCORES))
    ).results
    raw = np.concatenate([res[c]["OUT"] for c in range(NCORES)], axis=0)
    return post(raw, X, target)
